# revision 41
# baseline (speedup 1.0000x reference)
"""Trainium2 Bass kernel for the Mamba-style block nn_Block_29721173688983.

Strategy: tensor-parallel over d_inner (2048 channels / 8 cores = 256 each).
Each core: RMSNorm (redundant), its w_in column slice, depthwise conv, silu,
partial x_proj contraction -> on-device AllReduce (the only collective),
delta via softplus Taylor poly, selective scan via DVE tensor_tensor_scan
(n-major lane layout: 16 state dims x 2 channel blocks of 128, L chunked for
pipelining), y = sum_n via identity-matmul PSUM accumulation, out_proj
partial matmul.  Host sums the 8 partial outputs and adds the residual.

kernel(**inputs) takes the FULL unsharded inputs from setup_inputs() and
returns the FULL (1, 2048, 1024) output.
"""

import sys

sys.path.insert(0, "/opt/trn_rl_repo")

from contextlib import ExitStack

import numpy as np

import concourse.bacc as bacc
import concourse.bass as bass
import concourse.tile as tile
from concourse import mybir
from concourse.bass_utils import run_bass_kernel_spmd

F32 = mybir.dt.float32
F32R = mybir.dt.float32r
BF16 = mybir.dt.bfloat16
AF = mybir.ActivationFunctionType
OP = mybir.AluOpType

CORES = 8
D = 1024
DI = 2048
CL = DI // CORES          # 256 channels per core
NB = CL // 128            # 2 channel blocks
NST = 16                  # d_state
DTR = 64                  # dt_rank
KCONV = 4
EPS = 1e-5
LN2 = 0.6931471805599453


class _StopBuild(Exception):
    pass


def build(L=2048, collective=True, stop_after="D", reps=1, cskip=(),
          mul_mode="alldve", cbufs=3, ar_bf16=True, u_on_pool=True):
    """Emit the SPMD single-core program (same program on all 8 cores)."""
    LTS = min(512, L)          # psum free-dim tile
    NLT = L // LTS
    KB = D // 128              # 8 k-blocks for the w_in matmul
    LC = min(1024, L)          # phase-C scan chunk
    NCH = L // LC

    nc = bacc.Bacc("TRN2", target_bir_lowering=False, debug=False,
                   num_devices=CORES if collective else 1)

    def din(name, shape, dt=F32):
        return nc.dram_tensor(name, shape, dt, kind="ExternalInput").ap()

    xT_d = din("xT", [D, L], BF16)
    w_in_d = din("w_in_pack", [128, KB * 512], BF16)        # [p, (kb, m*128)]
    cw_d = din("cw_pack", [128, NB * KCONV])
    cbias_d = din("cbias_pack", [128, NB])
    A_d = din("A_pack", [128, NB * NST])
    D_d = din("D_pack", [128, NB])
    wxp_d = din("wxp_pack", [128, NB * (DTR + 2 * NST)], BF16)
    wdt_d = din("wdt_loc", [DTR, CL])
    wdtb_d = din("wdtb_loc", [DTR, CL], BF16)
    bdt_d = din("bdt_pack", [128, NB])
    wout_d = din("wout_pack", [128, NB * D], BF16)
    ident_d = din("ident", [128, 128], F32R)
    identb_d = din("ident_bf", [128, 128], BF16)
    ones_d = din("ones_in", [128, 1], F32R)
    cwdiag_d = din("cwdiag_pack", [128, NB * KCONV * 128], BF16)

    pout_d = nc.dram_tensor("part_out", [D, L], F32, kind="ExternalOutput").ap()

    NPROJ = DTR + 2 * NST      # 96

    with tile.TileContext(nc) as tc:
      with ExitStack() as ctx:
        try:
            # ---- persistent pools ----
            cpool = ctx.enter_context(tc.tile_pool(name="consts", bufs=1))
            drpool = ctx.enter_context(
                tc.tile_pool(name="dram", bufs=1, space="DRAM"))

            cw_t = cpool.tile([128, NB * KCONV], F32)
            cbias_t = cpool.tile([128, NB], F32)
            A_t = cpool.tile([128, NB * NST], F32)
            Aneg_t = cpool.tile([128, NB * NST], F32)
            D_t = cpool.tile([128, NB], F32)
            wxp_t = cpool.tile([128, NB * NPROJ], BF16)
            wdt_t = cpool.tile([DTR, CL], F32)
            wdtb_t = cpool.tile([DTR, CL], BF16)
            bdt_t = cpool.tile([128, NB], F32)
            wout_t = cpool.tile([128, NB * D], BF16)
            ident_t = cpool.tile([128, 128], F32R)
            identb_t = cpool.tile([128, 128], BF16)
            ones_t = cpool.tile([128, 1], F32R)
            cwdiag_t = cpool.tile([128, NB * KCONV * 128], BF16)

            nc.sync.dma_start(cw_t[:], cw_d[:])
            nc.sync.dma_start(cbias_t[:], cbias_d[:])
            nc.sync.dma_start(A_t[:], A_d[:])
            nc.sync.dma_start(D_t[:], D_d[:])
            nc.sync.dma_start(wxp_t[:], wxp_d[:])
            nc.sync.dma_start(wdt_t[:], wdt_d[:])
            nc.sync.dma_start(wdtb_t[:], wdtb_d[:])
            nc.sync.dma_start(bdt_t[:], bdt_d[:])
            nc.sync.dma_start(wout_t[:], wout_d[:])
            nc.sync.dma_start(ident_t[:], ident_d[:])
            nc.sync.dma_start(identb_t[:], identb_d[:])
            nc.sync.dma_start(ones_t[:], ones_d[:])
            nc.sync.dma_start(cwdiag_t[:], cwdiag_d[:])

            # A = -exp(A_log)
            nc.scalar.activation(Aneg_t[:], A_t[:], AF.Exp)
            nc.scalar.mul(Aneg_t[:], Aneg_t[:], -1.0)

            for rep in range(reps):
              with ExitStack() as rctx:
                # per-rep persistent activations (live through phase C)
                ppool = rctx.enter_context(
                    tc.tile_pool(name=f"persist{rep}", bufs=1))
                ARDT = BF16 if ar_bf16 else F32
                ar_in = drpool.tile([NPROJ, L], ARDT, name=f"ar_in{rep}")
                ar_out = drpool.tile([NPROJ, L], ARDT, addr_space="Shared",
                                     name=f"ar_out{rep}")
                rinv_dram = drpool.tile([1, L], F32, name=f"rinv_dram{rep}")
                res_silu = [ppool.tile([128, L], F32,
                                       name=f"res_silu{i}_{rep}")
                            for i in range(NB)]

                # xs_pad + rinv_bc live only through A/B: right-side pool
                padpool_ctx = ExitStack()
                padpool = padpool_ctx.enter_context(
                    tc.tile_pool(name=f"pad{rep}", bufs=1, side="right"))
                xs_pad = [padpool.tile([128, L + KCONV - 1], BF16,
                                       name=f"xs_pad{i}_{rep}")
                          for i in range(NB)]
                rinv_bc = padpool.tile([128, L], F32, name=f"rinv_bc{rep}")
                for cb in range(NB):
                    nc.gpsimd.memset(xs_pad[cb][:, 0:KCONV - 1], 0.0)

                # ================= Phase A =================
                # Load xT; sum x^2 via ones-matmul; rinv via Newton rsqrt; main
                # matmul on UNSCALED xT (starts as soon as each block lands),
                # rinv applied on PSUM evacuation.
                with ExitStack() as actx:
                    wpool = actx.enter_context(tc.tile_pool(name="w_in", bufs=1))
                    xtpool = actx.enter_context(tc.tile_pool(name="xt", bufs=KB))
                    sqpool = actx.enter_context(tc.tile_pool(name="sq", bufs=2))
                    sspool = actx.enter_context(
                        tc.tile_pool(name="ps_ss", bufs=NLT, space="PSUM"))
                    mmpool = actx.enter_context(
                        tc.tile_pool(name="ps_mm", bufs=4, space="PSUM"))
                    rowpool = actx.enter_context(tc.tile_pool(name="rows", bufs=1))
                    evpool = actx.enter_context(tc.tile_pool(name="ev", bufs=3))

                    w_in_t = wpool.tile([128, KB * 512], BF16)

                    ss_ps = [sspool.tile([1, LTS], F32, tag="ss", name=f"ss{i}_{rep}")
                             for i in range(NLT)]
                    xt_ts = []
                    half = L // 2
                    for kb in range(KB):
                        xt = xtpool.tile([128, L], BF16, tag="xt")
                        nc.sync.dma_start(xt[:, 0:half],
                                          xT_d[bass.ts(kb, 128), 0:half])
                        nc.sync.dma_start(xt[:, half:L],
                                          xT_d[bass.ts(kb, 128), half:L])
                        xt_ts.append(xt)
                        sq = sqpool.tile([128, L], F32R, tag="sq")
                        nc.scalar.square(sq[:], xt[:])
                        for lt in range(NLT):
                            nc.tensor.matmul(
                                ss_ps[lt][:], ones_t[:],
                                sq[:, bass.ts(lt, LTS)],
                                start=(kb == 0), stop=(kb == KB - 1))

                    nc.sync.dma_start(w_in_t[:, 0:KB * 256], w_in_d[:, 0:KB * 256])
                    nc.sync.dma_start(w_in_t[:, KB * 256:], w_in_d[:, KB * 256:])

                    # rinv = rsqrt(ss/D + EPS): 2 Newton iters on [128, L/128]
                    rinv_row = rowpool.tile([1, L], F32)
                    for lt in range(NLT):
                        nc.scalar.activation(
                            rinv_row[:, bass.ts(lt, LTS)], ss_ps[lt][:],
                            AF.Copy, bias=EPS, scale=1.0 / D)
                    nc.sync.dma_start(rinv_dram[:], rinv_row[:])
                    LF = L // 128
                    m_t = rowpool.tile([128, LF], F32)
                    nc.sync.dma_start(
                        m_t[:], rinv_dram[:].rearrange("a (p f) -> (a p) f", p=128))
                    y_t = rowpool.tile([128, LF], F32)
                    t_t = rowpool.tile([128, LF], F32)
                    nc.vector.tensor_scalar(y_t[:], m_t[:], -0.5, 1.5,
                                            op0=OP.mult, op1=OP.add)
                    for _ in range(2):
                        nc.vector.tensor_mul(t_t[:], y_t[:], y_t[:])
                        nc.vector.tensor_mul(t_t[:], t_t[:], m_t[:])
                        nc.vector.tensor_scalar(t_t[:], t_t[:], -0.5, 1.5,
                                                op0=OP.mult, op1=OP.add)
                        nc.vector.tensor_mul(y_t[:], y_t[:], t_t[:])
                    nc.sync.dma_start(
                        rinv_dram[:].rearrange("a (p f) -> (a p) f", p=128), y_t[:])
                    nc.sync.dma_start(rinv_bc[:],
                                      rinv_dram[:].partition_broadcast(128))

                    # main matmul on raw xT; scale by rinv on evacuation.
                    for m in range(2 * NB):
                        for lt in range(NLT):
                            mm_ps = mmpool.tile([128, LTS], F32, tag="mm")
                            for kb in range(KB):
                                nc.tensor.matmul(
                                    mm_ps[:],
                                    w_in_t[:, kb * 512 + m * 128:
                                           kb * 512 + (m + 1) * 128],
                                    xt_ts[kb][:, bass.ts(lt, LTS)],
                                    start=(kb == 0), stop=(kb == KB - 1))
                            if m < NB:
                                nc.vector.tensor_mul(
                                    xs_pad[m][:, KCONV - 1 + lt * LTS:
                                              KCONV - 1 + (lt + 1) * LTS],
                                    mm_ps[:], rinv_bc[:, bass.ts(lt, LTS)])
                            else:
                                ev = evpool.tile([128, LTS], F32, tag="ev")
                                nc.vector.tensor_mul(
                                    ev[:], mm_ps[:], rinv_bc[:, bass.ts(lt, LTS)])
                                nc.scalar.activation(
                                    res_silu[m - NB][:, bass.ts(lt, LTS)], ev[:],
                                    AF.Silu)

                if stop_after == "A":
                    padpool_ctx.close()
                    continue

                # ================= Phase B =================
                apool = rctx.enter_context(tc.tile_pool(name=f"acts{rep}", bufs=1))
                xs_silu = [apool.tile([128, L], BF16, name=f"xs_silu{i}_{rep}")
                           for i in range(NB)]
                delta = [apool.tile([128, L], F32, name=f"delta{i}_{rep}")
                         for i in range(NB)]
                u_bf = [apool.tile([128, L], BF16, name=f"u{i}_{rep}")
                        for i in range(NB)]
                bcb_dram = None if ar_bf16 else drpool.tile(
                    [2 * NST, L], BF16, name=f"bcb{rep}")
                with ExitStack() as bctx:
                    convpool = bctx.enter_context(tc.tile_pool(name="conv", bufs=2))
                    prpool = bctx.enter_context(tc.tile_pool(name="proj", bufs=1))
                    pspool = bctx.enter_context(
                        tc.tile_pool(name="ps_b", bufs=2, space="PSUM"))

                    # depthwise causal conv on PE: 4 PSUM-accumulated
                    # diagonal matmuls per (cb, lt); Silu on evacuation.
                    for cb in range(NB):
                        for lt in range(NLT):
                            c_ps = pspool.tile([128, LTS], F32, tag="cps")
                            for j in range(KCONV):
                                nc.tensor.matmul(
                                    c_ps[:],
                                    cwdiag_t[:, (cb * KCONV + j) * 128:
                                             (cb * KCONV + j + 1) * 128],
                                    xs_pad[cb][:, j + lt * LTS:
                                               j + lt * LTS + LTS],
                                    start=(j == 0), stop=(j == KCONV - 1))
                            nc.scalar.activation(
                                xs_silu[cb][:, bass.ts(lt, LTS)], c_ps[:],
                                AF.Silu, bias=cbias_t[:, cb:cb + 1])

                    padpool_ctx.close()

                    proj_sb = prpool.tile([NPROJ, L], ARDT)
                    for lt in range(NLT):
                        pr_ps = pspool.tile([NPROJ, LTS], F32, tag="prps")
                        for cb in range(NB):
                            nc.tensor.matmul(
                                pr_ps[:],
                                wxp_t[:, cb * NPROJ:(cb + 1) * NPROJ],
                                xs_silu[cb][:, bass.ts(lt, LTS)],
                                start=(cb == 0), stop=(cb == NB - 1))
                        nc.scalar.copy(proj_sb[:, bass.ts(lt, LTS)], pr_ps[:])

                    nc.sync.dma_start(ar_in[:], proj_sb[:])
                    if collective:
                        nc.gpsimd.collective_compute(
                            "AllReduce", OP.add,
                            replica_groups=[list(range(CORES))],
                            ins=[ar_in.opt()], outs=[ar_out.opt()])
                    else:
                        nc.sync.dma_start(ar_out[:], ar_in[:])

                    # delta path.  If ar_bf16, the B/C rows come back from
                    # the collective already bf16 so the per-n broadcasts
                    # read ar_out directly; otherwise stage a bf16 copy.
                    pd_sb = prpool.tile([DTR, L], ARDT)
                    nc.sync.dma_start(pd_sb[:], ar_out[0:DTR, :])
                    if not ar_bf16:
                        bc_sb = prpool.tile([2 * NST, L], F32)
                        nc.sync.dma_start(bc_sb[:], ar_out[DTR:NPROJ, :])
                        bcb_sb = prpool.tile([2 * NST, L], BF16)
                        nc.scalar.copy(bcb_sb[:], bc_sb[:])
                        nc.sync.dma_start(bcb_dram[:], bcb_sb[:])
                    # softplus(z) = ln(exp(z + b_dt) + 1): two Act ops, both
                    # in the natural_log_exp table (shared with phase C Exp).
                    wdt_use = wdtb_t if ar_bf16 else wdt_t
                    for cb in range(NB):
                        ez = prpool.tile([128, L], F32, name=f"ez{cb}_{rep}",
                                         tag="ez")
                        for lt in range(NLT):
                            ls = bass.ts(lt, LTS)
                            d_ps = pspool.tile([128, LTS], F32, tag="dps")
                            nc.tensor.matmul(
                                d_ps[:], wdt_use[:, bass.ts(cb, 128)],
                                pd_sb[:, ls],
                                start=True, stop=True)
                            nc.scalar.activation(
                                ez[:, ls], d_ps[:],
                                AF.Exp, bias=bdt_t[:, cb:cb + 1])
                            nc.scalar.activation(
                                delta[cb][:, ls], ez[:, ls],
                                AF.Ln, bias=1.0)
                        if u_on_pool:
                            nc.gpsimd.tensor_tensor(
                                u_bf[cb][:], delta[cb][:],
                                xs_silu[cb][:], op=OP.mult)
                        else:
                            nc.vector.tensor_mul(
                                u_bf[cb][:], delta[cb][:], xs_silu[cb][:])

                if stop_after == "B":
                    continue

                # ================= Phase C: scan =================
                # Lanes: [128 channels] x (n in 16, cb in 2, chunk in NCH).
                ypool_ctx = ExitStack()
                ypool = ypool_ctx.enter_context(
                    tc.tile_pool(name="ps_y", bufs=NB * NLT, space="PSUM"))
                y_ps = [[ypool.tile([128, LTS], F32, tag="yps",
                                    name=f"yps{cb}_{lt}_{rep}")
                         for lt in range(NLT)] for cb in range(NB)]
                if "ymm" in cskip:
                    for cb in range(NB):
                        for lt in range(NLT):
                            nc.vector.memset(y_ps[cb][lt][:], 0.0)

                with ExitStack() as cctx:
                    bcpool = cctx.enter_context(tc.tile_pool(name="bc", bufs=3))
                    scpool = cctx.enter_context(
                        tc.tile_pool(name="scw", bufs=cbufs))
                    zpool = cctx.enter_context(
                        tc.tile_pool(name="zw", bufs=cbufs))

                    # Engine split for the 64 [128,L] muls: DVE ~1.8us each
                    # but also owns the 32 scans (~5.6us each); Pool ~5.2us.
                    if mul_mode == "alldve":
                        def mul_engine(k):
                            return nc.vector
                    elif mul_mode == "dvedbx":
                        def mul_engine(k):
                            return nc.vector if k % 2 == 0 else nc.gpsimd
                    else:
                        def mul_engine(k):
                            return nc.vector if k % 3 == 0 else nc.gpsimd

                    bc_src = ar_out if ar_bf16 else bcb_dram
                    bc_off = DTR if ar_bf16 else 0
                    for n in range(NST):
                        Bb = bcpool.tile([128, L], BF16, tag="Bb")
                        Cb = bcpool.tile([128, L], BF16, tag="Cb")
                        nc.sync.dma_start(
                            Bb[:], bc_src[bc_off + n:bc_off + n + 1, :]
                            .partition_broadcast(128))
                        nc.sync.dma_start(
                            Cb[:], bc_src[bc_off + NST + n:bc_off + NST + n + 1, :]
                            .partition_broadcast(128))
                        for cb in range(NB):
                            idx = n * NB + cb
                            if "exp" in cskip:
                                da = delta[cb][:]
                            else:
                                da_t = scpool.tile([128, L], F32, tag="da")
                                nc.scalar.activation(
                                    da_t[:], delta[cb][:], AF.Exp,
                                    scale=Aneg_t[:, cb * NST + n:cb * NST + n + 1])
                                da = da_t[:]
                            if "mul1" in cskip:
                                dbx = u_bf[cb][:]
                            else:
                                dbx_t = scpool.tile([128, L], BF16, tag="dbxz")
                                mul_engine(2 * idx).tensor_tensor(
                                    dbx_t[:], u_bf[cb][:], Bb[:], op=OP.mult)
                                dbx = dbx_t[:]
                            if "scan" in cskip:
                                ys = dbx
                            else:
                                ys_t = scpool.tile([128, L], BF16, tag="ys")
                                nc.vector.tensor_tensor_scan(
                                    ys_t[:], da, dbx, 0.0,
                                    op0=OP.mult, op1=OP.add)
                                ys = ys_t[:]
                            if "mul2" in cskip:
                                z = ys
                            else:
                                z_t = zpool.tile([128, L], BF16, tag="z")
                                # Tail z-muls go to Pool: they complete at
                                # phase-C end anyway, so they cannot delay
                                # the next rep's collective (also on Pool),
                                # and each one frees ~1.8us of DVE.
                                zeng = (nc.gpsimd if idx >= 26
                                        else mul_engine(2 * idx + 1))
                                zeng.tensor_tensor(
                                    z_t[:], ys, Cb[:], op=OP.mult)
                                z = z_t[:]
                            if "ymm" not in cskip:
                                for lt in range(NLT):
                                    nc.tensor.matmul(
                                        y_ps[cb][lt][:], identb_t[:],
                                        z[:, bass.ts(lt, LTS)],
                                        start=(n == 0), stop=(n == NST - 1))

                # ====== consume y psum into fin ======
                fpool = rctx.enter_context(tc.tile_pool(name=f"fin{rep}", bufs=1))
                fin = [fpool.tile([128, L], BF16, name=f"fin{i}_{rep}")
                       for i in range(NB)]
                for cb in range(NB):
                    for lt in range(NLT):
                        nc.vector.scalar_tensor_tensor(
                            fin[cb][:, bass.ts(lt, LTS)],
                            xs_silu[cb][:, bass.ts(lt, LTS)],
                            D_t[:, cb:cb + 1], y_ps[cb][lt][:],
                            op0=OP.mult, op1=OP.add)
                    nc.vector.tensor_mul(fin[cb][:], fin[cb][:],
                                         res_silu[cb][:])
                ypool_ctx.close()

                if stop_after == "C":
                    continue

                # ============= Phase D: out projection =============
                with ExitStack() as dctx:
                    opool = dctx.enter_context(tc.tile_pool(name="po", bufs=3))
                    opspool = dctx.enter_context(
                        tc.tile_pool(name="ps_o", bufs=3, space="PSUM"))

                    for m in range(D // 128):
                        po_sb = opool.tile([128, L], F32, tag="po")
                        for lt in range(NLT):
                            o_ps = opspool.tile([128, LTS], F32, tag="ops")
                            for cb in range(NB):
                                nc.tensor.matmul(
                                    o_ps[:],
                                    wout_t[:, cb * D + m * 128:
                                           cb * D + (m + 1) * 128],
                                    fin[cb][:, bass.ts(lt, LTS)],
                                    start=(cb == 0), stop=(cb == NB - 1))
                            nc.scalar.copy(po_sb[:, bass.ts(lt, LTS)], o_ps[:])
                        nc.sync.dma_start(pout_d[bass.ts(m, 128), :], po_sb[:])
        except _StopBuild:
            pass

    nc.compile()
    return nc


def host_prep(inputs, L=2048):
    """Slice/replicate the full inputs into 8 per-core input maps."""
    x = np.asarray(inputs["x"], np.float32)
    norm_scale = np.asarray(inputs["norm_scale"], np.float32)
    w_in = np.asarray(inputs["w_in"], np.float32)
    conv_w = np.asarray(inputs["conv_w"], np.float32)
    conv_b = np.asarray(inputs["conv_b"], np.float32)
    A_log = np.asarray(inputs["A_log"], np.float32)
    D_in = np.asarray(inputs["D"], np.float32)
    w_xproj = np.asarray(inputs["w_xproj"], np.float32)
    w_dt = np.asarray(inputs["w_dt"], np.float32)
    b_dt = np.asarray(inputs["b_dt"], np.float32)
    w_out = np.asarray(inputs["w_out"], np.float32)

    import ml_dtypes

    x2 = x[0, :L, :]                              # (L, D)
    xT = np.ascontiguousarray(x2.T)               # (D, L)
    w_in_s = w_in * norm_scale[:, None]
    ident = np.eye(128, dtype=np.float32)
    ident_bf = np.eye(128, dtype=ml_dtypes.bfloat16)
    KB = D // 128

    def pack_nb(v):                                # (CL,) -> [128, NB]
        return np.ascontiguousarray(v.reshape(NB, 128).T)

    in_maps = []
    for k in range(CORES):
        sl = slice(k * CL, (k + 1) * CL)
        wi = np.concatenate(
            [w_in_s[:, k * CL:(k + 1) * CL],
             w_in_s[:, DI + k * CL:DI + (k + 1) * CL]], axis=1)  # (D, 512)
        w_in_pack = np.ascontiguousarray(
            wi.reshape(KB, 128, 512).transpose(1, 0, 2).reshape(128, KB * 512))
        cw = conv_w[:, 0, sl]                     # (4, CL)
        cw_pack = np.ascontiguousarray(
            cw.reshape(KCONV, NB, 128).transpose(2, 1, 0)
            .reshape(128, NB * KCONV))
        A_pack = np.ascontiguousarray(
            A_log[sl].reshape(NB, 128, NST).transpose(1, 0, 2)
            .reshape(128, NB * NST))
        wxp_pack = np.ascontiguousarray(
            w_xproj[sl].reshape(NB, 128, DTR + 2 * NST)
            .transpose(1, 0, 2).reshape(128, NB * (DTR + 2 * NST)))
        wout_pack = np.ascontiguousarray(
            w_out[sl].reshape(NB, 128, D).transpose(1, 0, 2)
            .reshape(128, NB * D))
        in_maps.append({
            "xT": xT.astype(ml_dtypes.bfloat16),
            "w_in_pack": w_in_pack.astype(ml_dtypes.bfloat16),
            "cw_pack": cw_pack,
            "cbias_pack": pack_nb(conv_b[sl]),
            "A_pack": A_pack,
            "D_pack": pack_nb(D_in[sl]),
            "wxp_pack": wxp_pack.astype(ml_dtypes.bfloat16),
            "wdt_loc": np.ascontiguousarray(w_dt[:, sl]),
            "wdtb_loc": np.ascontiguousarray(w_dt[:, sl]).astype(
                ml_dtypes.bfloat16),
            "bdt_pack": pack_nb(b_dt[sl]),
            "wout_pack": wout_pack.astype(ml_dtypes.bfloat16),
            "ident": ident,
            "ident_bf": ident_bf,
            "ones_in": np.ones((128, 1), np.float32),
            "cwdiag_pack": np.concatenate(
                [np.diag(cw[j, cb * 128:(cb + 1) * 128]).astype(np.float32)
                 for cb in range(NB) for j in range(KCONV)],
                axis=1).astype(ml_dtypes.bfloat16),
        })
    return in_maps


def combine(inputs, results, L=2048):
    """Host unshard: sum the 8 partial outputs, add residual."""
    x = np.asarray(inputs["x"], np.float32)
    acc = np.zeros((D, L), np.float32)
    for r in results:
        acc += r["part_out"]
    out = x[0, :L, :] + acc.T
    return out[None].astype(np.float32)


_CACHE = {}


def kernel(**inputs):
    if "nc" not in _CACHE:
        _CACHE["nc"] = build()
    nc = _CACHE["nc"]
    in_maps = host_prep(inputs)
    res = run_bass_kernel_spmd(nc, in_maps, list(range(CORES)))
    return combine(inputs, res.results)


if __name__ == "__main__":
    import reference

    inputs = reference.setup_inputs()
    inputs = {k: np.asarray(v) for k, v in inputs.items()}
    expected = np.asarray(reference.reference(**inputs))
    actual = kernel(**inputs)
    err = np.abs(actual - expected).max() / np.abs(expected).max()
    print("Relative error:", err)

